# revision 17
# baseline (speedup 1.0000x reference)
"""Self-contained TRN2 Bass kernel for axial attention (nn_AxialAttention).

kernel(**inputs) takes FULL inputs (x [8,128,128,512], Wq/Wk/Wv/Wo [512,512],
bo [512]) and returns the FULL output [8,128,128,512] (float32).

Strategy: data-parallel over N across 8 NeuronCores (core c computes image c).
Per core: fp32r projections/output matmul, bf16 attention middle, softmax
without max-subtraction (logits ~N(0,1)), PE transposes for X^T and A^T,
engine-balanced copies (ACT=exp only, DVE=psum exits, per-head accum sums).
"""
import sys
sys.path.insert(0, "/opt/trn_rl_repo")
sys.path.insert(0, "/root/.axon_site/_ro/trn_rl_repo")

"""Axial attention Bass kernel for TRN2 — builder shared by test.py and kernel.py.

Problem: x [N=8, H=128, W=128, C=512], attention along H (8 heads, head dim 64):
  per (n, w): seq = x[n, :, w, :] [128, 512]
  q/k/v = seq @ W{q,k,v}.T ; per head S = q k^T/8 ; A = softmax_j(S) ; out = A v
  O = out @ Wo.T + bo  -> out[n, :, w, :]

Sharding: data-parallel over N — core c computes image n=c entirely.

Per-core layout strategy (tokens t = h, seqs s = w, 4 seqs per block):
  X_blk  [128 t, 4 s, 512 c]   <- DMA (f32r)
  X^T    [128 c_loc, 4 jc, 4 s, 128 t]  via PE transpose (f32r)
  Q^T/K^T [128 co_loc, 4 co, 4 s, 128 t] = W^T.T @ X^T  (f32r matmul, bf16 out)
  V      [128 t, 4 s, 512 c]  (bf16 out)
  S_g    [128 i, 128 j] psum = Q^T_g.T @ K^T_g  (bf16)
  A      exp(S/8) -> [128 i, 8 g, 128 j] bf16 + per-g rowsum (ACT accum)
  AN     A * (1/rowsum) broadcast  (DVE)
  A^T    per-head DMA xbar transpose -> [128 j, 8 g, 128 i] bf16
  outT   [128 c_loc, 4 jc, 128 i] psum = V_g.T...: lhsT=V_g, rhs=A^T_g
  O      [128 t, 512 co] = outT.T @ Wo^T (f32r) + bo
"""
import numpy as np

import concourse.bass as bass
import concourse.bacc as bacc
import concourse.tile as tile
from concourse import mybir

F32 = mybir.dt.float32
F32R = mybir.dt.float32r
BF16 = mybir.dt.bfloat16
EXP = mybir.ActivationFunctionType.Exp

H = 128   # tokens per sequence (attention axis)
W = 128   # sequences per core
C = 512
G = 8     # heads
GP = C // G  # 64
BLK = 4   # sequences per block
NBLK = W // BLK
NCHUNK = C // 128  # 4 k-chunks


def build_kernel(num_cores=8, attn_f32=False, w_total=W, reps=1, an_engine='dve', at_engine='dve', psum_bufs=(3,2,3), exp_mode='perhead', attn_mode='std', sbufs=None, psum_unified=False):
    """Build + compile the Bass module. Returns nc.

    reps>1 wraps the whole computation in a dynamic loop (for timing by
    wall-clock differencing; results are identical, just recomputed)."""
    nblk = w_total // BLK
    nc = bacc.Bacc("TRN2", target_bir_lowering=False, debug=False,
                   num_devices=num_cores)

    x_d = nc.dram_tensor("x", [H, w_total, C], F32R, kind="ExternalInput").ap()
    wq_d = nc.dram_tensor("wqT", [C, C], F32R, kind="ExternalInput").ap()
    wk_d = nc.dram_tensor("wkT", [C, C], F32R, kind="ExternalInput").ap()
    wv_d = nc.dram_tensor("wvT", [C, C], F32R, kind="ExternalInput").ap()
    wo_d = nc.dram_tensor("woT", [C, C], F32R, kind="ExternalInput").ap()
    bo_d = nc.dram_tensor("bo", [C], F32, kind="ExternalInput").ap()
    id_d = nc.dram_tensor("ident", [128, 128], F32R, kind="ExternalInput").ap()
    out_d = nc.dram_tensor("out", [H, w_total, C], F32, kind="ExternalOutput").ap()

    AMID = F32R if attn_f32 else BF16  # dtype of attention middle section

    sb_bufs = sbufs or {}
    def B(name, d):
        return sb_bufs.get(name, d)
    with tile.TileContext(nc) as tc:
        with tc.tile_pool(name="consts", bufs=1) as consts, \
             tc.tile_pool(name="px", bufs=B('px', 2)) as px, \
             tc.tile_pool(name="pxt", bufs=B('pxt', 2)) as pxt, \
             tc.tile_pool(name="pqt", bufs=B('pqt', 2)) as pqt, \
             tc.tile_pool(name="pv", bufs=B('pv', 2)) as pv, \
             tc.tile_pool(name="pa", bufs=B('pa', 3)) as pa, \
             tc.tile_pool(name="pstat", bufs=B('pstat', 4)) as pstat, \
             tc.tile_pool(name="pot", bufs=B('pot', 2)) as pot, \
             tc.tile_pool(name="po", bufs=B('po', 2)) as po, \
             tc.tile_pool(name="pdram", bufs=6, space="DRAM") as pdram, \
             tc.tile_pool(name="psf", bufs=(8 if psum_unified else psum_bufs[0]), space="PSUM") as psf, \
             tc.tile_pool(name="pss", bufs=psum_bufs[1], space="PSUM") as _pss, \
             tc.tile_pool(name="psb", bufs=psum_bufs[2], space="PSUM") as _psb:
            if psum_unified:
                class _U:
                    _n = [0]
                    def tile(self, shape, dtype, tag=None):
                        self._n[0] += 1
                        return psf.tile(shape, dtype, tag="f",
                                        name=f"u{self._n[0]}")
                pss = psb = _U()
            else:
                pss, psb = _pss, _psb

            # ---- constants ----
            wq_sb = consts.tile([128, NCHUNK, C], F32R, tag="wq")
            wk_sb = consts.tile([128, NCHUNK, C], F32R, tag="wk")
            wv_sb = consts.tile([128, NCHUNK, C], F32R, tag="wv")
            wo_sb = consts.tile([128, NCHUNK, C], F32R, tag="wo")
            for w_sb, w_d in ((wq_sb, wq_d), (wk_sb, wk_d), (wv_sb, wv_d),
                              (wo_sb, wo_d)):
                nc.sync.dma_start(w_sb[:], w_d.rearrange("(j p) c -> p j c", p=128))
            bo_sb = consts.tile([128, C], F32, tag="bo")
            nc.sync.dma_start(
                bo_sb[:],
                bo_d.rearrange("(o c) -> o c", o=1).broadcast_to((128, C)))
            id_sb = consts.tile([128, 128], F32R, tag="id")
            nc.sync.dma_start(id_sb[:], id_d[:])
            id_bf = consts.tile([128, 128], BF16, tag="idbf")
            nc.vector.tensor_copy(id_bf[:], id_sb[:].bitcast(F32))
            ones_bf = consts.tile([128, 128], BF16, tag="ones")
            nc.vector.memset(ones_bf[:], 1.0)

            state = {}

            def front(b):
                X_blk = px.tile([128, BLK, C], F32R, tag="x")
                nc.sync.dma_start(X_blk[:], x_d[:, b * BLK:(b + 1) * BLK, :])
                XT_sb = pxt.tile([128, NCHUNK, BLK, 128], F32R, tag="xt")
                for s in range(BLK):
                    XT_ps = psf.tile([128, NCHUNK, 128], F32R, tag="f")
                    for jc in range(NCHUNK):
                        nc.tensor.transpose(
                            XT_ps[:, jc, :],
                            X_blk[:, s, jc * 128:(jc + 1) * 128], id_sb[:])
                    nc.vector.tensor_copy(XT_sb[:, :, s, :], XT_ps[:])
                QT = pqt.tile([128, NCHUNK, BLK, 128], AMID, tag="qt")
                KT = pqt.tile([128, NCHUNK, BLK, 128], AMID, tag="kt")
                for w_sb, dst in ((wq_sb, QT), (wk_sb, KT)):
                    for co in range(NCHUNK):
                        PT = psf.tile([128, BLK * 128], F32, tag="f")
                        for jc in range(NCHUNK):
                            nc.tensor.matmul(
                                PT[:],
                                lhsT=w_sb[:, jc, co * 128:(co + 1) * 128],
                                rhs=XT_sb[:, jc, :, :],
                                start=(jc == 0), stop=(jc == NCHUNK - 1))
                        nc.vector.tensor_copy(dst[:, co, :, :], PT[:])
                V = pv.tile([128, BLK, C], AMID, tag="v")
                for s in range(BLK):
                    VP = psf.tile([128, C], F32, tag="f")
                    for jc in range(NCHUNK):
                        nc.tensor.matmul(
                            VP[:], lhsT=XT_sb[:, jc, s, :],
                            rhs=wv_sb[:, jc, :],
                            start=(jc == 0), stop=(jc == NCHUNK - 1))
                    nc.vector.tensor_copy(V[:, s, :], VP[:])
                state[b] = (QT, KT, V)

            def back(b):
                QT, KT, V = state.pop(b)
                O_sb = po.tile([128, BLK, C], F32, tag="o")
                for s in range(BLK):
                    A = pa.tile([128, G, 128], AMID, tag="a")
                    sums = pstat.tile([128, G], F32, tag="sums")
                    # Even heads (PE row-group 0) and odd heads (row-group 1)
                    # run concurrently in the array -> MUST land in different
                    # PSUM banks (same-bank concurrent row-group writes hang).
                    S_e = pss.tile([128, G // 2, 128], F32, tag="s")
                    S_o = pss.tile([128, G // 2, 128], F32, tag="s")
                    for g in range(G):
                        p0 = 64 * (g % 2)
                        S_ps = S_e if g % 2 == 0 else S_o
                        nc.tensor.matmul(
                            S_ps[:, g // 2, :],
                            lhsT=QT[p0:p0 + 64, g // 2, s, :],
                            rhs=KT[p0:p0 + 64, g // 2, s, :],
                            start=True, stop=True)
                    if exp_mode == 'perhead':
                        for g in range(G):
                            S_ps = S_e if g % 2 == 0 else S_o
                            nc.scalar.activation(
                                A[:, g, :], S_ps[:, g // 2, :], EXP,
                                scale=1.0 / np.sqrt(GP),
                                accum_out=sums[:, g:g + 1])
                    else:
                        # batched: 2 exps; A slices by even/odd heads; sums via
                        # DVE reduce over [128, 4, 128] per half
                        a_ev = A[:, 0:G:2, :]
                        a_od = A[:, 1:G:2, :]
                        nc.scalar.activation(a_ev, S_e[:], EXP,
                                             scale=1.0 / np.sqrt(GP))
                        nc.scalar.activation(a_od, S_o[:], EXP,
                                             scale=1.0 / np.sqrt(GP))
                        se = sums[:, 0:G:2].rearrange("p (g o) -> p g o", o=1)
                        so = sums[:, 1:G:2].rearrange("p (g o) -> p g o", o=1)
                        nc.vector.reduce_sum(se, a_ev, axis=mybir.AxisListType.X)
                        nc.vector.reduce_sum(so, a_od, axis=mybir.AxisListType.X)
                    rcp = pstat.tile([128, G], F32, tag="rcp")
                    nc.vector.reciprocal(rcp[:], sums[:])
                    AN = pa.tile([128, G, 128], AMID, tag="an")
                    rcp_b = rcp[:].rearrange("p (o g) -> p o g", o=1) \
                        .rearrange("p o g -> p g o") \
                        .broadcast_to((128, G, 128))
                    if an_engine == 'pool':
                        nc.gpsimd.tensor_mul(AN[:], A[:], rcp_b)
                    else:
                        nc.vector.tensor_mul(AN[:], A[:], rcp_b)
                    # A^T via PE transposes (xbar DMA transpose races with
                    # concurrent DRAM DMA traffic on this HW/runtime).
                    AT = pa.tile([128, G, 128], AMID, tag="at")
                    idt = id_bf if AMID == BF16 else id_sb
                    for half in range(2):
                        ATp = psb.tile([128, G // 2, 128], AMID, tag="b")
                        for gg in range(G // 2):
                            g = half * (G // 2) + gg
                            nc.tensor.transpose(ATp[:, gg, :], AN[:, g, :],
                                                idt[:])
                        eng = nc.vector if at_engine == 'dve' else nc.scalar
                        if at_engine == 'dve':
                            nc.vector.tensor_copy(
                                AT[:, half * (G // 2):(half + 1) * (G // 2), :],
                                ATp[:])
                        else:
                            nc.scalar.copy(
                                AT[:, half * (G // 2):(half + 1) * (G // 2), :],
                                ATp[:])
                    OT_ps = psb.tile([128, NCHUNK, 128], F32, tag="b")
                    for g in range(G):
                        p0 = 64 * (g % 2)
                        nc.tensor.matmul(
                            OT_ps[p0:p0 + 64, g // 2, :],
                            lhsT=V[:, s, 64 * g:64 * (g + 1)],
                            rhs=AT[:, g, :],
                            start=True, stop=True)
                    OT_sb = pot.tile([128, NCHUNK, 128], F32R, tag="ot")
                    nc.vector.tensor_copy(OT_sb[:], OT_ps[:])
                    O_ps = psb.tile([128, C], F32, tag="b")
                    for jc in range(NCHUNK):
                        nc.tensor.matmul(
                            O_ps[:], lhsT=OT_sb[:, jc, :],
                            rhs=wo_sb[:, jc, :],
                            start=(jc == 0), stop=(jc == NCHUNK - 1))
                    nc.vector.tensor_add(O_sb[:, s, :], O_ps[:], bo_sb[:])
                nc.sync.dma_start(out_d[:, b * BLK:(b + 1) * BLK, :], O_sb[:])

            def back_st(b):
                # S^T-direct attention: exp(S^T) IS A^T (no transposes, no
                # normalization multiply on A). Softmax sums via GpSimd
                # cross-partition reduce; normalization folded into the
                # OT psum->sbuf copy using a reciprocal tile broadcast
                # through a DRAM scratch.
                QT, KT, V = state.pop(b)
                O_sb = po.tile([128, BLK, C], F32, tag="o")
                for s in range(BLK):
                    AT = pa.tile([128, G, 128], AMID, tag="at")
                    S_e = pss.tile([128, G // 2, 128], F32, tag="s")
                    S_o = pss.tile([128, G // 2, 128], F32, tag="s")
                    for g in range(G):
                        p0 = 64 * (g % 2)
                        S_ps = S_e if g % 2 == 0 else S_o
                        # lhsT=K^T, rhs=Q^T  ->  S^T[j, i]
                        nc.tensor.matmul(
                            S_ps[:, g // 2, :],
                            lhsT=KT[p0:p0 + 64, g // 2, s, :],
                            rhs=QT[p0:p0 + 64, g // 2, s, :],
                            start=True, stop=True)
                    if exp_mode == 'perhead':
                        for g in range(G):
                            S_ps = S_e if g % 2 == 0 else S_o
                            nc.scalar.activation(
                                AT[:, g, :], S_ps[:, g // 2, :], EXP,
                                scale=1.0 / np.sqrt(GP))
                    else:
                        nc.scalar.activation(AT[:, 0:G:2, :], S_e[:], EXP,
                                             scale=1.0 / np.sqrt(GP))
                        nc.scalar.activation(AT[:, 1:G:2, :], S_o[:], EXP,
                                             scale=1.0 / np.sqrt(GP))
                    # sums[g, i] = sum_j A^T[j, g, i]  (partition reduce)
                    sums = pstat.tile([1, G, 128], F32, tag="sums")
                    nc.gpsimd.reduce_sum(sums[:], AT[:],
                                         axis=mybir.AxisListType.C)
                    # broadcast sums to [128, jc, i]: row (2jc + p//64)
                    sc_d = pdram.tile([1, G * 128], F32, tag="scr")
                    nc.sync.dma_start(sc_d[:], sums[:])
                    st = pstat.tile([128, NCHUNK, 128], F32, tag="st")
                    s4 = sc_d[:].rearrange("o (jc hg i) -> (o hg) jc i",
                                           hg=2, i=128)
                    for hg in range(2):
                        src = s4[hg:hg + 1].broadcast_to((64, NCHUNK, 128))
                        nc.sync.dma_start(st[hg * 64:(hg + 1) * 64, :, :], src)
                    rcp_t = pstat.tile([128, NCHUNK, 128], F32, tag="rcpt")
                    nc.vector.reciprocal(rcp_t[:], st[:])
                    OT_ps = psb.tile([128, NCHUNK, 128], F32, tag="b")
                    for g in range(G):
                        p0 = 64 * (g % 2)
                        nc.tensor.matmul(
                            OT_ps[p0:p0 + 64, g // 2, :],
                            lhsT=V[:, s, 64 * g:64 * (g + 1)],
                            rhs=AT[:, g, :],
                            start=True, stop=True)
                    OT_sb = pot.tile([128, NCHUNK, 128], F32R, tag="ot")
                    nc.vector.tensor_mul(OT_sb[:], OT_ps[:], rcp_t[:])
                    O_ps = psb.tile([128, C], F32, tag="b")
                    for jc in range(NCHUNK):
                        nc.tensor.matmul(
                            O_ps[:], lhsT=OT_sb[:, jc, :],
                            rhs=wo_sb[:, jc, :],
                            start=(jc == 0), stop=(jc == NCHUNK - 1))
                    nc.vector.tensor_add(O_sb[:, s, :], O_ps[:], bo_sb[:])
                nc.sync.dma_start(out_d[:, b * BLK:(b + 1) * BLK, :], O_sb[:])

            def back_st2(b):
                # S^T-direct: exp(S^T) IS A^T. Softmax sums via all-ones
                # matmul (broadcasts column sums to every partition in PSUM);
                # normalization folded into the OT psum->sbuf copy.
                QT, KT, V = state.pop(b)
                O_sb = po.tile([128, BLK, C], F32, tag="o")
                for s in range(BLK):
                    AT = pa.tile([128, G, 128], AMID, tag="at")
                    S_e = pss.tile([128, G // 2, 128], F32, tag="s")
                    S_o = pss.tile([128, G // 2, 128], F32, tag="s")
                    for g in range(G):
                        p0 = 64 * (g % 2)
                        S_ps = S_e if g % 2 == 0 else S_o
                        nc.tensor.matmul(
                            S_ps[:, g // 2, :],
                            lhsT=KT[p0:p0 + 64, g // 2, s, :],
                            rhs=QT[p0:p0 + 64, g // 2, s, :],
                            start=True, stop=True)
                    if exp_mode == 'perhead':
                        for g in range(G):
                            S_ps = S_e if g % 2 == 0 else S_o
                            nc.scalar.activation(
                                AT[:, g, :], S_ps[:, g // 2, :], EXP,
                                scale=1.0 / np.sqrt(GP))
                    else:
                        nc.scalar.activation(AT[:, 0:G:2, :], S_e[:], EXP,
                                             scale=1.0 / np.sqrt(GP))
                        nc.scalar.activation(AT[:, 1:G:2, :], S_o[:], EXP,
                                             scale=1.0 / np.sqrt(GP))
                    # column sums broadcast to all partitions:
                    # bc_e[p, g*128+i] = sum_j AT[j, g, i]  (heads 0-3)
                    bc_e = psb.tile([128, 512], F32, tag="b")
                    bc_o = psb.tile([128, 512], F32, tag="b")
                    nc.tensor.matmul(bc_e[:], lhsT=ones_bf[:],
                                     rhs=AT[:, 0:4, :], start=True, stop=True)
                    nc.tensor.matmul(bc_o[:], lhsT=ones_bf[:],
                                     rhs=AT[:, 4:8, :], start=True, stop=True)
                    # rcp_t[p, jc, i] = 1/sums[2jc + p//64, i]
                    rcp_t = pstat.tile([128, NCHUNK, 128], F32, tag="rcpt")
                    for hg in range(2):
                        pr = slice(hg * 64, (hg + 1) * 64)
                        for hc, bc in ((0, bc_e), (1, bc_o)):
                            # [64, 2, 128] view: cols hg*128 + jc*256
                            src = bc[pr, :].rearrange(
                                "p (jc r i) -> p jc r i", jc=2, i=128)[
                                :, :, hg, :]
                            nc.vector.reciprocal(
                                rcp_t[pr, 2 * hc:2 * hc + 2, :], src)
                    OT_ps = psb.tile([128, NCHUNK, 128], F32, tag="b")
                    for g in range(G):
                        p0 = 64 * (g % 2)
                        nc.tensor.matmul(
                            OT_ps[p0:p0 + 64, g // 2, :],
                            lhsT=V[:, s, 64 * g:64 * (g + 1)],
                            rhs=AT[:, g, :],
                            start=True, stop=True)
                    OT_sb = pot.tile([128, NCHUNK, 128], F32R, tag="ot")
                    nc.vector.tensor_mul(OT_sb[:], OT_ps[:], rcp_t[:])
                    O_ps = psb.tile([128, C], F32, tag="b")
                    for jc in range(NCHUNK):
                        nc.tensor.matmul(
                            O_ps[:], lhsT=OT_sb[:, jc, :],
                            rhs=wo_sb[:, jc, :],
                            start=(jc == 0), stop=(jc == NCHUNK - 1))
                    nc.vector.tensor_add(O_sb[:, s, :], O_ps[:], bo_sb[:])
                nc.sync.dma_start(out_d[:, b * BLK:(b + 1) * BLK, :], O_sb[:])

            back_fn = {'st': back_st, 'st2': back_st2}.get(attn_mode, back)

            def whole():
                for b in range(nblk + 1):
                    if b < nblk:
                        front(b)
                    if b >= 1:
                        back_fn(b - 1)

            if reps == 1:
                whole()
            else:
                with tc.For_i(0, reps, 1):
                    whole()

    nc.compile()
    return nc


def build_kernel_v3(num_cores=8, reps=1, psum_bufs=(2, 4, 0, 2), sbufs=None,
                    assign=None):
    """v3: all-bf16 datapath, S^T-direct attention, masked-ones softmax sums,
    normalization folded into the OT psum exit, exits balanced DVE/ACT, X
    f32->bf16 conversion on the idle GPSIMD (Pool) engine.

    Per core (image): x [H=128 tokens, W=128 seqs, C=512]; 8 heads of 64.
    Per block b (BLK=4 seqs):
      front: X dma -> Xb bf16 (pool) -> X^T via PE transpose (bf16)
             -> QT/KT = W^T.T @ X^T (bf16, [co, t]) ; V = X @ Wv^T ([t, co])
      back:  per s: S^T = KT^T QT (8 MMs, 64-contraction, even/odd row grps)
             A^T = exp(S^T/8) on ACT (2 batched, strided head slots)
             rsum[p,jc,i] = colsum(A^T) via 2 masked-ones MMs (p<64: even
               heads, p>=64: odd) -> rcp_t = 1/rsum (DVE)
             OT = V^T A^T (8 MMs) ; OT_sb = OT * rcp_t (DVE, normalize)
             O = OT^T Wo^T (4 MMs) -> exit (ACT). bias bo added host-side.
    """
    nc = bacc.Bacc("TRN2", target_bir_lowering=False, debug=False,
                   num_devices=num_cores)

    x_d = nc.dram_tensor("x", [H, W, C], F32R, kind="ExternalInput").ap()
    wq_d = nc.dram_tensor("wqT", [C, C], F32, kind="ExternalInput").ap()
    wk_d = nc.dram_tensor("wkT", [C, C], F32, kind="ExternalInput").ap()
    wv_d = nc.dram_tensor("wvT", [C, C], F32, kind="ExternalInput").ap()
    wo_d = nc.dram_tensor("woT", [C, C], F32, kind="ExternalInput").ap()
    id_d = nc.dram_tensor("ident", [128, 128], F32R, kind="ExternalInput").ap()
    out_d = nc.dram_tensor("out", [H, W, C], F32, kind="ExternalOutput").ap()

    asg = {"xt": "dve", "qt": "dve", "kt": "act", "v": "act", "o": "act",
           "otmul": "dve", "xconv": "pool", "exp_batched": True,
           "skip_norm": False, "skip_attn": False, "skip_rcp": False,
           "rcp_fast": True, "pipelined": True}
    asg.update(assign or {})

    def eng(name):
        return {"dve": nc.vector, "act": nc.scalar, "pool": nc.gpsimd}[
            asg[name]]

    def exit_copy(name, dst, src):
        e = eng(name)
        if e is nc.scalar:
            e.copy(dst, src)
        else:
            e.tensor_copy(dst, src)

    sb_bufs = sbufs or {}

    def B(name, d):
        return sb_bufs.get(name, d)

    with tile.TileContext(nc) as tc:
        with tc.tile_pool(name="consts", bufs=1) as consts, \
             tc.tile_pool(name="pw", bufs=2) as pw, \
             tc.tile_pool(name="px", bufs=B('px', 3)) as px, \
             tc.tile_pool(name="pxb", bufs=B('pxb', 3)) as pxb, \
             tc.tile_pool(name="pxt", bufs=B('pxt', 3)) as pxt, \
             tc.tile_pool(name="pqt", bufs=B('pqt', 3)) as pqt, \
             tc.tile_pool(name="pv", bufs=B('pv', 3)) as pv, \
             tc.tile_pool(name="pa", bufs=B('pa', 4)) as pa, \
             tc.tile_pool(name="pstat", bufs=B('pstat', 4)) as pstat, \
             tc.tile_pool(name="pot", bufs=B('pot', 3)) as pot, \
             tc.tile_pool(name="po", bufs=B('po', 3)) as po, \
             tc.tile_pool(name="psf", bufs=psum_bufs[0], space="PSUM") as psf, \
             tc.tile_pool(name="pss", bufs=psum_bufs[1], space="PSUM") as pss, \
             tc.tile_pool(name="psr", bufs=max(1, psum_bufs[2]),
                          space="PSUM") as psr, \
             tc.tile_pool(name="psb", bufs=psum_bufs[3], space="PSUM") as psb:

            # ---- constants ----
            # weights: dma fp32 staging -> bf16 sbuf
            w_bfs = {}
            for wname, w_d in (("wq", wq_d), ("wk", wk_d), ("wv", wv_d),
                               ("wo", wo_d)):
                stage = pw.tile([128, NCHUNK, C], F32, tag="stage")
                nc.sync.dma_start(stage[:],
                                  w_d.rearrange("(j p) c -> p j c", p=128))
                w_bf = consts.tile([128, NCHUNK, C], BF16, tag=wname)
                nc.vector.tensor_copy(w_bf[:], stage[:])
                w_bfs[wname] = w_bf
            wq_sb, wk_sb, wv_sb, wo_sb = (w_bfs[k] for k in
                                          ("wq", "wk", "wv", "wo"))
            id_sb = consts.tile([128, 128], F32R, tag="id")
            nc.sync.dma_start(id_sb[:], id_d[:])
            id_bf = consts.tile([128, 128], BF16, tag="idbf")
            nc.vector.tensor_copy(id_bf[:], id_sb[:].bitcast(F32))
            ones_bf = consts.tile([128, 64], BF16, tag="ones")
            nc.vector.memset(ones_bf[:], 1.0)

            state = {}

            def front(b):
                X_blk = px.tile([128, BLK, C], F32R, tag="x")
                nc.sync.dma_start(X_blk[:], x_d[:, b * BLK:(b + 1) * BLK, :])
                if asg["xconv"] is not None:
                    Xb = pxb.tile([128, BLK, C], BF16, tag="xb")
                    eng("xconv").tensor_copy(Xb[:], X_blk[:].bitcast(F32))
                    xin, idt, tdt = Xb, id_bf, BF16
                else:
                    xin, idt, tdt = X_blk, id_sb, F32R
                XT_sb = pxt.tile([128, NCHUNK, BLK, 128], BF16, tag="xt")
                for s in range(BLK):
                    XT_ps = psf.tile([128, NCHUNK, 128], tdt, tag="f")
                    for jc in range(NCHUNK):
                        nc.tensor.transpose(
                            XT_ps[:, jc, :],
                            xin[:, s, jc * 128:(jc + 1) * 128], idt[:])
                    exit_copy("xt", XT_sb[:, :, s, :], XT_ps[:])
                QT = pqt.tile([128, NCHUNK, BLK, 128], BF16, tag="qt")
                KT = pqt.tile([128, NCHUNK, BLK, 128], BF16, tag="kt")
                for w_sb, dst, ex in ((wq_sb, QT, "qt"), (wk_sb, KT, "kt")):
                    for co in range(NCHUNK):
                        PT = psf.tile([128, BLK * 128], F32, tag="f")
                        for jc in range(NCHUNK):
                            nc.tensor.matmul(
                                PT[:],
                                lhsT=w_sb[:, jc, co * 128:(co + 1) * 128],
                                rhs=XT_sb[:, jc, :, :],
                                start=(jc == 0), stop=(jc == NCHUNK - 1))
                        exit_copy(ex, dst[:, co, :, :], PT[:])
                V = pv.tile([128, BLK, C], BF16, tag="v")
                for s in range(BLK):
                    VP = psf.tile([128, C], F32, tag="f")
                    for jc in range(NCHUNK):
                        nc.tensor.matmul(
                            VP[:], lhsT=XT_sb[:, jc, s, :],
                            rhs=wv_sb[:, jc, :],
                            start=(jc == 0), stop=(jc == NCHUNK - 1))
                    exit_copy("v", V[:, s, :], VP[:])
                state[b] = (QT, KT, V)

            def back(b):
                QT, KT, V = state.pop(b)
                O_sb = po.tile([128, BLK, C], F32, tag="o")
                st = [dict() for _ in range(BLK)]

                def st1(s):
                    # S^T matmuls + exp -> A^T (unnormalized)
                    AT = pa.tile([128, G, 128], BF16, tag="at")
                    S_e = pss.tile([128, G // 2, 128], F32, tag="s")
                    S_o = pss.tile([128, G // 2, 128], F32, tag="s")
                    for g in range(G):
                        p0 = 64 * (g % 2)
                        S_ps = S_e if g % 2 == 0 else S_o
                        # lhsT=K^T, rhs=Q^T -> S^T[j, i]; even/odd row groups
                        # must land in different PSUM banks.
                        nc.tensor.matmul(
                            S_ps[:, g // 2, :],
                            lhsT=KT[p0:p0 + 64, g // 2, s, :],
                            rhs=QT[p0:p0 + 64, g // 2, s, :],
                            start=True, stop=True)
                    if asg["exp_batched"]:
                        nc.scalar.activation(AT[:, 0:G:2, :], S_e[:], EXP,
                                             scale=1.0 / np.sqrt(GP))
                        nc.scalar.activation(AT[:, 1:G:2, :], S_o[:], EXP,
                                             scale=1.0 / np.sqrt(GP))
                    else:
                        for g in range(G):
                            S_ps = S_e if g % 2 == 0 else S_o
                            nc.scalar.activation(
                                AT[:, g, :], S_ps[:, g // 2, :], EXP,
                                scale=1.0 / np.sqrt(GP))
                    st[s]["AT"] = AT

                def st2(s):
                    # rsum[p, jc, i] = sum_j A^T[j, head, i]; p<64 holds even
                    # heads (head=2jc), p>=64 odd (head=2jc+1) -- matches the
                    # OT psum layout, so normalization is one tensor_mul.
                    if asg["skip_norm"]:
                        return
                    AT = st[s]["AT"]
                    rsum = pss.tile([128, NCHUNK, 128], F32, tag="s")
                    nc.tensor.matmul(rsum[0:64, :, :], lhsT=ones_bf[:],
                                     rhs=AT[:, 0:G:2, :], start=True,
                                     stop=True)
                    nc.tensor.matmul(rsum[64:128, :, :], lhsT=ones_bf[:],
                                     rhs=AT[:, 1:G:2, :], start=True,
                                     stop=True)
                    rcp_t = pstat.tile([128, NCHUNK, 128], F32, tag="rcpt")
                    if asg["skip_rcp"]:
                        nc.vector.tensor_copy(rcp_t[:], rsum[:])
                    elif asg["rcp_fast"]:
                        nc.vector.reciprocal_approx_fast(rcp_t[:], rsum[:])
                    else:
                        nc.vector.reciprocal(rcp_t[:], rsum[:])
                    st[s]["rcp_t"] = rcp_t

                def st3(s):
                    AT = st[s]["AT"]
                    OT_ps = psb.tile([128, NCHUNK, 128], F32, tag="b")
                    for g in range(G):
                        p0 = 64 * (g % 2)
                        nc.tensor.matmul(
                            OT_ps[p0:p0 + 64, g // 2, :],
                            lhsT=V[:, s, 64 * g:64 * (g + 1)],
                            rhs=AT[:, g, :],
                            start=True, stop=True)
                    OT_sb = pot.tile([128, NCHUNK, 128], BF16, tag="ot")
                    if asg["skip_norm"]:
                        eng("otmul").tensor_copy(OT_sb[:], OT_ps[:])
                    else:
                        eng("otmul").tensor_mul(OT_sb[:], OT_ps[:],
                                                st[s]["rcp_t"][:])
                    st[s]["OT_sb"] = OT_sb

                def st4(s):
                    OT_sb = st[s]["OT_sb"]
                    O_ps = psb.tile([128, C], F32, tag="b")
                    for jc in range(NCHUNK):
                        nc.tensor.matmul(
                            O_ps[:], lhsT=OT_sb[:, jc, :],
                            rhs=wo_sb[:, jc, :],
                            start=(jc == 0), stop=(jc == NCHUNK - 1))
                    exit_copy("o", O_sb[:, s, :], O_ps[:])

                if asg["pipelined"]:
                    # software pipeline: dependencies cross >=1 stage of
                    # other-iteration PE work, so the in-order PE never
                    # waits on a just-issued ACT/DVE result.
                    for t in range(BLK + 2):
                        if t < BLK:
                            st1(t)
                        if 1 <= t <= BLK:
                            st2(t - 1)
                            st3(t - 1)
                        if t >= 2:
                            st4(t - 2)
                else:
                    for s in range(BLK):
                        st1(s)
                        st2(s)
                        st3(s)
                        st4(s)
                nc.sync.dma_start(out_d[:, b * BLK:(b + 1) * BLK, :], O_sb[:])

            def whole():
                for b in range(NBLK + 1):
                    if b < NBLK:
                        front(b)
                    if b >= 1:
                        back(b - 1)

            if reps == 1:
                whole()
            else:
                with tc.For_i(0, reps, 1):
                    whole()

    nc.compile()
    return nc


def make_in_maps(x, Wq, Wk, Wv, Wo, bo, num_cores=8, include_bo=True):
    """Full inputs -> per-core input dicts (data-parallel over N)."""
    x = np.asarray(x, dtype=np.float32)
    ident = np.eye(128, dtype=np.float32)
    wqT = np.ascontiguousarray(np.asarray(Wq, np.float32).T)
    wkT = np.ascontiguousarray(np.asarray(Wk, np.float32).T)
    wvT = np.ascontiguousarray(np.asarray(Wv, np.float32).T)
    woT = np.ascontiguousarray(np.asarray(Wo, np.float32).T)
    bo = np.asarray(bo, np.float32)
    maps = [{"x": np.ascontiguousarray(x[n]), "wqT": wqT, "wkT": wkT,
             "wvT": wvT, "woT": woT, "bo": bo, "ident": ident}
            for n in range(num_cores)]
    if not include_bo:
        for m in maps:
            del m["bo"]
    return maps


_NC_CACHE = {}


def kernel(x, Wq, Wk, Wv, Wo, bo):
    import numpy as np
    from concourse import bass_utils

    if "nc" not in _NC_CACHE:
        _NC_CACHE["nc"] = build_kernel(
            num_cores=8, attn_mode="std", psum_bufs=(2, 3, 3),
            sbufs={"px": 3, "pxt": 3, "pqt": 3, "pv": 3, "pa": 4,
                   "pstat": 6, "pot": 3, "po": 3})
    nc = _NC_CACHE["nc"]
    in_maps = make_in_maps(x, Wq, Wk, Wv, Wo, bo, num_cores=8)
    res = bass_utils.run_bass_kernel_spmd(nc, in_maps, core_ids=list(range(8)))
    out = np.stack([res.results[c]["out"] for c in range(8)], axis=0)
    return out.astype(np.float32)

